# revision 1
# baseline (speedup 1.0000x reference)
"""Bass/Trainium2 kernel for nn_BespokeEmbedding (moe_routing).

Strategy (data-parallel over tokens across 8 NeuronCores):
  - Host computes per-token category codes (cat_table[token_ids]) and routes
    the 32768 tokens into per-category groups split evenly across the cores
    (the dispatch step of the expert routing; any core can serve any token
    since tables are replicated), gathering each group's embedding rows into
    a contraction-major fp16 activation block, pre-packed into the SBUF
    partition layout so every device DMA is a contiguous 128-descriptor
    transfer.
  - Each core runs one Bass/Tile kernel: for every category (smallest first,
    streamed just-in-time in need order on one DMA queue), a dense fp16
    matmul Y_c^T = W_c^T @ X_c^T accumulated over 128-row K tiles in PSUM,
    with one stationary-weight load serving both 512-token chunks, bias-add
    fused into the PSUM drain (split across Vector and Scalar engines),
    result streamed back as Y_c^T [D, M_PAD] fp16 via the GpSimd DMA path
    (last category via the by-then-idle sync HWDGE for a faster exit).
  - Host scatters rows back to token order (inverse of the dispatch) and
    returns the full [8, 4096, 1024] float32 output.

fp16 runs the PE at 1 cycle/row (4x the fp32 rate) and, unlike
fp32/float32r, its weight loads overlap in-flight matmuls; end-to-end
relative error is ~4e-4 (fp16 input/output rounding; PSUM accumulates in
fp32). Measured ~109 us HW exec per core; the matmul roofline for this
decomposition is ~90 us.
"""

import numpy as np

B, S, V, D = 8, 4096, 50257, 1024
CAT_DIMS = (1536, 1024, 512, 256)
NAMES = ("high", "mid", "low", "special")
N_CORES = 8
TOK_PER_CORE = (B * S) // N_CORES      # 4096
M_PAD = 1024                           # per-core per-category padded group size
CHUNKS = ((0, 512), (512, 512))        # token chunks of M_PAD
N_DCOL = D // 128                      # 8
ORDER = ("special", "low", "mid", "high")      # smallest tables first

_CACHE = {}
LAST_EXEC_NS = None
LAST_RESULTS = None


def _build_bass():
    from contextlib import ExitStack
    import concourse.bacc as bacc
    import concourse.mybir as mybir
    import concourse.tile as tile

    nc = bacc.Bacc("TRN2", target_bir_lowering=False, debug=False,
                   num_devices=N_CORES)
    f16 = mybir.dt.float16
    f32 = mybir.dt.float32
    ident = mybir.ActivationFunctionType.Identity
    dims = dict(zip(NAMES, CAT_DIMS))

    xt_d, w_d, yt_d = {}, {}, {}
    for nm in NAMES:
        nk = dims[nm] // 128
        # inputs come pre-packed in SBUF partition layout
        xt_d[nm] = nc.dram_tensor(f"xt_{nm}", [128, nk * M_PAD], f16,
                                  kind="ExternalInput")
        w_d[nm] = nc.dram_tensor(f"w_{nm}", [128, nk * D], f16,
                                 kind="ExternalInput")
        yt_d[nm] = nc.dram_tensor(f"yt_{nm}", [D, M_PAD], f16,
                                  kind="ExternalOutput")
    # bias packed host-side as [128, 4*8]: column c*8+j holds b_c[j*128:(j+1)*128]
    bias_d = nc.dram_tensor("bias", [128, len(NAMES) * N_DCOL], f32,
                            kind="ExternalInput")

    with tile.TileContext(nc) as tc, ExitStack() as ctx:
        wpool = ctx.enter_context(tc.tile_pool(name="w", bufs=1))
        xpool = ctx.enter_context(tc.tile_pool(name="x", bufs=4))
        opool = ctx.enter_context(tc.tile_pool(name="o", bufs=16))
        bpool = ctx.enter_context(tc.tile_pool(name="b", bufs=1))
        ppool = ctx.enter_context(tc.tile_pool(name="p", bufs=3, space="PSUM"))

        bias_t = bpool.tile([128, len(NAMES) * N_DCOL], f32)

        # PE warm-up: ~3.5us of dummy matmuls on a zeroed tile while the first
        # real inputs stream in, so the HAM clock-gate is released (2.4 GHz)
        # by the time real matmuls issue.
        warm = bpool.tile([128, 640], f16, name="warm")
        nc.vector.memset(warm[:], 0.0)
        wps = ppool.tile([128, 512], f32, tag="warmps", name="warmps", bufs=1)
        for r in range(16):
            nc.tensor.matmul(wps[:], warm[:, :128], warm[:, 128:640],
                             start=(r == 0), stop=(r == 15))

        w_t = {}
        for nm in ORDER:
            ci = NAMES.index(nm)
            nk = dims[nm] // 128
            # just-in-time, need-ordered input streaming on the sync queue:
            # weights for this category, then its activation block
            w_t[nm] = wpool.tile([128, nk * D], f16, tag=f"w_{nm}",
                                 name=f"w_{nm}_sb")
            x_t = xpool.tile([128, 12 * M_PAD], f16, tag="xslab", name=f"x_{nm}")
            if nm == ORDER[0]:
                # first category: finest granularity so the first j-loop's
                # matmuls start as soon as each k-slab lands
                for k in range(nk):
                    nc.sync.dma_start(w_t[nm][:, k * D:(k + 1) * D],
                                      w_d[nm].ap()[:, k * D:(k + 1) * D])
                    nc.sync.dma_start(
                        x_t[:, k * M_PAD:(k + 1) * M_PAD],
                        xt_d[nm].ap()[:, k * M_PAD:(k + 1) * M_PAD])
                nc.sync.dma_start(bias_t[:], bias_d.ap())
            else:
                nc.sync.dma_start(w_t[nm][:], w_d[nm].ap())
                half = (nk // 2) * M_PAD
                nc.sync.dma_start(x_t[:, :half], xt_d[nm].ap()[:, :half])
                nc.sync.dma_start(x_t[:, half:nk * M_PAD],
                                  xt_d[nm].ap()[:, half:])
            for j in range(N_DCOL):
                pss = [ppool.tile([128, 512], f32, tag=f"acc{q}", name=f"ps{q}")
                       for q in range(len(CHUNKS))]
                for k in range(nk):
                    # one stationary load of W[k-block, j-block] serves both
                    # token chunks
                    for q, (c0, n) in enumerate(CHUNKS):
                        nc.tensor.matmul(
                            pss[q][:, :n],
                            w_t[nm][:, k * D + j * 128: k * D + (j + 1) * 128],
                            x_t[:, k * M_PAD + c0: k * M_PAD + c0 + n],
                            start=(k == 0),
                            stop=(k == nk - 1),
                        )
                o_t = opool.tile([128, M_PAD], f16, tag="ostage")
                bias_ap = bias_t[:, ci * N_DCOL + j: ci * N_DCOL + j + 1]
                # split the PSUM drain across two engines so it never paces PE
                nc.vector.tensor_scalar_add(o_t[:, 0:512], pss[0][:, :512], bias_ap)
                nc.scalar.activation(o_t[:, 512:1024], pss[1][:, :512], ident,
                                     bias=bias_ap)
                out_eng = nc.sync if nm == ORDER[-1] else nc.gpsimd
                out_eng.dma_start(yt_d[nm].ap()[j * 128:(j + 1) * 128, :], o_t[:])
    nc.compile()
    return nc


def _get_nc():
    if "nc" not in _CACHE:
        _CACHE["nc"] = _build_bass()
    return _CACHE["nc"]


def _pack_sbuf_layout(a2d):
    """[nk*128, F] -> [128, nk*F] (SBUF partition-major, contiguous)."""
    nk = a2d.shape[0] // 128
    f = a2d.shape[1]
    return np.ascontiguousarray(
        a2d.reshape(nk, 128, f).transpose(1, 0, 2).reshape(128, nk * f)
    )


def kernel(_profile=False, **inputs):
    global LAST_EXEC_NS, LAST_RESULTS
    from concourse.bass_utils import run_bass_kernel_spmd

    token_ids = np.asarray(inputs["token_ids"]).astype(np.int64)
    cat_table = np.asarray(inputs["cat_table"]).astype(np.int64)
    emb = {nm: np.asarray(inputs[f"emb_{nm}"], dtype=np.float32) for nm in NAMES}
    W = {nm: np.asarray(inputs[f"W_{nm}"], dtype=np.float32) for nm in NAMES}
    bvec = {nm: np.asarray(inputs[f"b_{nm}"], dtype=np.float32) for nm in NAMES}

    W16 = {nm: _pack_sbuf_layout(W[nm].astype(np.float16)) for nm in NAMES}
    bias_packed = np.ascontiguousarray(
        np.concatenate([bvec[nm].reshape(N_DCOL, 128).T for nm in NAMES], axis=1),
        dtype=np.float32)

    tok_flat = token_ids.reshape(-1)          # [32768]
    cats = cat_table[tok_flat]                # [32768]

    # Global routing: each category's token list is split evenly across the 8
    # cores (any core can serve any token -- tables are replicated), so every
    # group is exactly <= M_PAD with no per-core variance. The rare global
    # excess beyond 8*M_PAD per category falls back to the host.
    groups = {}     # (core, nm) -> global token positions
    overflow = []   # (nm, global positions beyond total capacity)
    for ci, nm in enumerate(NAMES):
        pos = np.nonzero(cats == ci)[0]
        if len(pos) > N_CORES * M_PAD:
            overflow.append((nm, pos[N_CORES * M_PAD:]))
            pos = pos[:N_CORES * M_PAD]
        for core in range(N_CORES):
            groups[(core, nm)] = pos[core * M_PAD:(core + 1) * M_PAD]

    in_maps = []
    for core in range(N_CORES):
        im = {"bias": bias_packed}
        for ci, (nm, d) in enumerate(zip(NAMES, CAT_DIMS)):
            pos = groups[(core, nm)]
            n = len(pos)
            X = np.zeros((M_PAD, d), np.float16)
            if n:
                X[:n] = emb[nm][tok_flat[pos]]
            # [M_PAD, d] -> K-major [d, M_PAD] -> SBUF layout [128, nk*M_PAD]
            nk = d // 128
            im[f"xt_{nm}"] = np.ascontiguousarray(
                X.reshape(M_PAD, nk, 128).transpose(2, 1, 0).reshape(128, nk * M_PAD)
            )
            im[f"w_{nm}"] = W16[nm]
        in_maps.append(im)

    nc = _get_nc()
    res = run_bass_kernel_spmd(nc, in_maps, list(range(N_CORES)),
                               trace=bool(_profile))
    LAST_EXEC_NS = res.exec_time_ns
    LAST_RESULTS = res

    out = np.empty((B * S, D), np.float32)
    for core in range(N_CORES):
        for nm in NAMES:
            pos = groups[(core, nm)]
            n = len(pos)
            if n:
                yt = res.results[core][f"yt_{nm}"]     # [D, M_PAD] fp16
                out[pos] = yt[:, :n].T.astype(np.float32)
    # rare global excess beyond 8*M_PAD tokens in one category: host fallback
    for nm, pos in overflow:
        rows = emb[nm][tok_flat[pos]]
        out[pos] = rows @ W[nm] + bvec[nm]

    return out.reshape(B, S, D)



# revision 2
# speedup vs baseline: 1.0647x; 1.0647x over previous
"""Bass/Trainium2 kernel for nn_BespokeEmbedding (moe_routing).

Strategy (unique-token data-parallel across 8 NeuronCores):
  - Host dedups the 32768 tokens to their ~24k unique ids (output rows are
    identical for repeated ids), routes the unique tokens into per-category
    groups split evenly across the cores, and gathers each group's embedding
    rows into a contraction-major fp16 activation block pre-packed into the
    SBUF partition layout. M_PAD (per-core per-category padded group size)
    is derived from the actual counts (~754 vs 1024 without dedup), so the
    matmul stream shrinks ~35%.
  - Each core runs one Bass/Tile kernel: for every category (smallest first,
    streamed just-in-time), a dense fp16 matmul Y_c^T = W_c^T @ X_c^T
    accumulated over 128-row K tiles in PSUM, one stationary-weight load
    serving both token chunks, bias-add fused into the PSUM drain (split
    across Vector and Scalar engines), result streamed back as fp16.
  - Inputs stream on two HWDGE queues in parallel (weights on sync, X on
    scalar) plus the tail half of W_high on the gpsimd queue, because the
    deduped compute (~65us) outruns a single ~290 GB/s input queue.
  - Host scatters unique rows back to all token positions (inverse of the
    dedup) and returns the full [8, 4096, 1024] float32 output.

fp16 runs the PE at 1 cycle/row; fp8 double-pumping was evaluated and
rejected: e4m3 quantization of E and W gives max rel err ~4.5e-2 against
the 2e-2 gate (verified numerically), and correction passes erase the 2x
rate gain. PSUM accumulates in fp32; end-to-end rel err ~4e-4.
"""

import numpy as np

B, S, V, D = 8, 4096, 50257, 1024
CAT_DIMS = (1536, 1024, 512, 256)
NAMES = ("high", "mid", "low", "special")
N_CORES = 8
N_DCOL = D // 128                      # 8
ORDER = ("special", "low", "mid", "high")      # smallest tables first
MAX_MP = 1024                          # SBUF cap; excess falls back to host

_CACHE = {}
LAST_EXEC_NS = None
LAST_RESULTS = None


def _build_bass(mp):
    from contextlib import ExitStack
    import concourse.bacc as bacc
    import concourse.mybir as mybir
    import concourse.tile as tile

    nc = bacc.Bacc("TRN2", target_bir_lowering=False, debug=False,
                   num_devices=N_CORES)
    f16 = mybir.dt.float16
    f32 = mybir.dt.float32
    ident = mybir.ActivationFunctionType.Identity
    dims = dict(zip(NAMES, CAT_DIMS))
    c0n = min(512, mp)
    c1n = mp - c0n
    chunks = [(0, c0n)] + ([(c0n, c1n)] if c1n else [])

    xt_d, w_d, yt_d = {}, {}, {}
    for nm in NAMES:
        nk = dims[nm] // 128
        # inputs come pre-packed in SBUF partition layout
        xt_d[nm] = nc.dram_tensor(f"xt_{nm}", [128, nk * mp], f16,
                                  kind="ExternalInput")
        w_d[nm] = nc.dram_tensor(f"w_{nm}", [128, nk * D], f16,
                                 kind="ExternalInput")
        yt_d[nm] = nc.dram_tensor(f"yt_{nm}", [D, mp], f16,
                                  kind="ExternalOutput")
    # bias packed host-side as [128, 4*8]: column c*8+j holds b_c[j*128:(j+1)*128]
    bias_d = nc.dram_tensor("bias", [128, len(NAMES) * N_DCOL], f32,
                            kind="ExternalInput")

    with tile.TileContext(nc) as tc, ExitStack() as ctx:
        wpool = ctx.enter_context(tc.tile_pool(name="w", bufs=1))
        xpool = ctx.enter_context(tc.tile_pool(name="x", bufs=4))
        opool = ctx.enter_context(tc.tile_pool(name="o", bufs=16))
        bpool = ctx.enter_context(tc.tile_pool(name="b", bufs=1))
        ppool = ctx.enter_context(tc.tile_pool(name="p", bufs=3, space="PSUM"))

        bias_t = bpool.tile([128, len(NAMES) * N_DCOL], f32)

        # PE warm-up: dummy matmuls on a zeroed tile while the first real
        # inputs stream in, releasing the HAM clock-gate (2.4 GHz by ~3us of
        # PE activity). Short because two input queues land data by ~9.5us.
        warm = bpool.tile([128, 640], f16, name="warm")
        nc.vector.memset(warm[:], 0.0)
        wps = ppool.tile([128, 512], f32, tag="warmps", name="warmps", bufs=1)
        for r in range(6):
            nc.tensor.matmul(wps[:], warm[:, :128], warm[:, 128:640],
                             start=(r == 0), stop=(r == 5))

        w_t, x_t = {}, {}
        for nm in ORDER:
            nk = dims[nm] // 128
            w_t[nm] = wpool.tile([128, nk * D], f16, tag=f"w_{nm}",
                                 name=f"w_{nm}_sb")
            x_t[nm] = xpool.tile([128, 12 * mp], f16, tag="xslab",
                                 name=f"x_{nm}")

        # Input streams, need-ordered. W on the sync HWDGE queue (first
        # category at k-block granularity so compute starts ASAP); X on the
        # scalar HWDGE queue; tail half of W_high on the gpsimd queue.
        nc.sync.dma_start(w_t["special"][:, :D], w_d["special"].ap()[:, :D])
        nc.sync.dma_start(w_t["special"][:, D:2 * D],
                          w_d["special"].ap()[:, D:2 * D])
        nc.sync.dma_start(bias_t[:], bias_d.ap())
        nc.sync.dma_start(w_t["low"][:], w_d["low"].ap())
        nc.sync.dma_start(w_t["mid"][:], w_d["mid"].ap())
        hk = 6 * D
        nc.sync.dma_start(w_t["high"][:, :hk], w_d["high"].ap()[:, :hk])
        nc.gpsimd.dma_start(w_t["high"][:, hk:12 * D],
                            w_d["high"].ap()[:, hk:])

        nc.scalar.dma_start(x_t["special"][:, :mp],
                            xt_d["special"].ap()[:, :mp])
        nc.scalar.dma_start(x_t["special"][:, mp:2 * mp],
                            xt_d["special"].ap()[:, mp:2 * mp])
        nc.scalar.dma_start(x_t["low"][:, :4 * mp], xt_d["low"].ap())
        nc.scalar.dma_start(x_t["mid"][:, :8 * mp], xt_d["mid"].ap())
        nc.scalar.dma_start(x_t["high"][:, :12 * mp], xt_d["high"].ap())

        for nm in ORDER:
            ci = NAMES.index(nm)
            nk = dims[nm] // 128
            for j in range(N_DCOL):
                pss = [ppool.tile([128, 512], f32, tag=f"acc{q}", name=f"ps{q}")
                       for q in range(len(chunks))]
                for k in range(nk):
                    # one stationary load of W[k-block, j-block] serves both
                    # token chunks
                    for q, (c0, n) in enumerate(chunks):
                        nc.tensor.matmul(
                            pss[q][:, :n],
                            w_t[nm][:, k * D + j * 128: k * D + (j + 1) * 128],
                            x_t[nm][:, k * mp + c0: k * mp + c0 + n],
                            start=(k == 0),
                            stop=(k == nk - 1),
                        )
                o_t = opool.tile([128, mp], f16, tag="ostage")
                bias_ap = bias_t[:, ci * N_DCOL + j: ci * N_DCOL + j + 1]
                # split the PSUM drain across two engines so it never paces PE
                nc.vector.tensor_scalar_add(o_t[:, :c0n], pss[0][:, :c0n],
                                            bias_ap)
                if c1n:
                    nc.scalar.activation(o_t[:, c0n:mp], pss[1][:, :c1n],
                                         ident, bias=bias_ap)
                r0, r1 = j * 128, (j + 1) * 128
                if nm != ORDER[-1]:
                    nc.gpsimd.dma_start(yt_d[nm].ap()[r0:r1, :], o_t[:])
                elif j < N_DCOL - 1 or not c1n:
                    # last category exits via the by-then-idle sync HWDGE
                    nc.sync.dma_start(yt_d[nm].ap()[r0:r1, :], o_t[:])
                else:
                    # tail: ship each chunk as soon as its drain lands
                    nc.sync.dma_start(yt_d[nm].ap()[r0:r1, :c0n], o_t[:, :c0n])
                    nc.sync.dma_start(yt_d[nm].ap()[r0:r1, c0n:mp],
                                      o_t[:, c0n:mp])
    nc.compile()
    return nc


def _get_nc(mp):
    if mp not in _CACHE:
        _CACHE[mp] = _build_bass(mp)
    return _CACHE[mp]


def _pack_sbuf_layout(a2d):
    """[nk*128, F] -> [128, nk*F] (SBUF partition-major, contiguous)."""
    nk = a2d.shape[0] // 128
    f = a2d.shape[1]
    return np.ascontiguousarray(
        a2d.reshape(nk, 128, f).transpose(1, 0, 2).reshape(128, nk * f)
    )


def kernel(_profile=False, **inputs):
    global LAST_EXEC_NS, LAST_RESULTS
    from concourse.bass_utils import run_bass_kernel_spmd

    token_ids = np.asarray(inputs["token_ids"]).astype(np.int64)
    cat_table = np.asarray(inputs["cat_table"]).astype(np.int64)
    emb = {nm: np.asarray(inputs[f"emb_{nm}"], dtype=np.float32) for nm in NAMES}
    W = {nm: np.asarray(inputs[f"W_{nm}"], dtype=np.float32) for nm in NAMES}
    bvec = {nm: np.asarray(inputs[f"b_{nm}"], dtype=np.float32) for nm in NAMES}

    W16 = {nm: _pack_sbuf_layout(W[nm].astype(np.float16)) for nm in NAMES}
    bias_packed = np.ascontiguousarray(
        np.concatenate([bvec[nm].reshape(N_DCOL, 128).T for nm in NAMES], axis=1),
        dtype=np.float32)

    tok_flat = token_ids.reshape(-1)          # [32768]
    uniq, inv = np.unique(tok_flat, return_inverse=True)
    ucats = cat_table[uniq]                   # [n_uniq]

    # Unique-token routing: each category's unique-token list is split evenly
    # across the 8 cores (tables are replicated). M_PAD is sized from the
    # actual per-category counts so there is no overflow for this input; a
    # host fallback guards pathological distributions that exceed MAX_MP.
    counts = [(ucats == ci).sum() for ci in range(len(NAMES))]
    mp = int(max(512 + 2, -(-max(counts) // N_CORES)))
    mp += mp % 2
    mp = min(mp, MAX_MP)

    groups = {}     # (core, nm) -> unique-token indices (into uniq)
    overflow = []   # (nm, unique-token indices beyond total capacity)
    for ci, nm in enumerate(NAMES):
        pos = np.nonzero(ucats == ci)[0]
        if len(pos) > N_CORES * mp:
            overflow.append((nm, pos[N_CORES * mp:]))
            pos = pos[:N_CORES * mp]
        for core in range(N_CORES):
            groups[(core, nm)] = pos[core * mp:(core + 1) * mp]

    in_maps = []
    for core in range(N_CORES):
        im = {"bias": bias_packed}
        for ci, (nm, d) in enumerate(zip(NAMES, CAT_DIMS)):
            pos = groups[(core, nm)]
            n = len(pos)
            X = np.zeros((mp, d), np.float16)
            if n:
                X[:n] = emb[nm][uniq[pos]]
            # [mp, d] -> K-major [d, mp] -> SBUF layout [128, nk*mp]
            nk = d // 128
            im[f"xt_{nm}"] = np.ascontiguousarray(
                X.reshape(mp, nk, 128).transpose(2, 1, 0).reshape(128, nk * mp)
            )
            im[f"w_{nm}"] = W16[nm]
        in_maps.append(im)

    nc = _get_nc(mp)
    res = run_bass_kernel_spmd(nc, in_maps, list(range(N_CORES)),
                               trace=bool(_profile))
    LAST_EXEC_NS = res.exec_time_ns
    LAST_RESULTS = res

    out_u = np.empty((len(uniq), D), np.float32)
    for core in range(N_CORES):
        for nm in NAMES:
            pos = groups[(core, nm)]
            n = len(pos)
            if n:
                yt = res.results[core][f"yt_{nm}"]     # [D, mp] fp16
                out_u[pos] = yt[:, :n].T.astype(np.float32)
    # pathological excess beyond 8*mp unique tokens in one category: host
    for nm, pos in overflow:
        rows = emb[nm][uniq[pos]]
        out_u[pos] = rows @ W[nm] + bvec[nm]

    return out_u[inv].reshape(B, S, D)


# revision 5
# speedup vs baseline: 1.1450x; 1.0754x over previous
"""Bass/Trainium2 kernel for nn_BespokeEmbedding (moe_routing).

Strategy (unique-token data-parallel across 8 NeuronCores):
  - Host dedups the 32768 tokens to their ~24k unique ids (output rows are
    identical for repeated ids), routes the unique tokens into per-category
    groups split evenly across the cores, and gathers each group's embedding
    rows into a contraction-major fp16 activation block pre-packed into the
    SBUF partition layout. M_PAD (per-core per-category padded group size)
    is derived from the actual counts (~754 vs 1024 without dedup), so the
    matmul stream shrinks ~35%.
  - Each core runs one Bass/Tile kernel: for every category (smallest first,
    streamed just-in-time), a dense fp16 matmul Y_c^T = W_c^T @ X_c^T
    accumulated over 128-row K tiles in PSUM, one stationary-weight load
    serving both token chunks, bias-add fused into the PSUM drain (split
    across Vector and Scalar engines), result streamed back as fp16.
  - Inputs stream on two HWDGE queues in parallel (weights on sync, X on
    scalar) plus the tail half of W_high on the gpsimd queue, because the
    deduped compute (~65us) outruns a single ~290 GB/s input queue.
  - Host scatters unique rows back to all token positions (inverse of the
    dedup) and returns the full [8, 4096, 1024] float32 output.

fp16 runs the PE at 1 cycle/row; fp8 double-pumping was evaluated and
rejected: e4m3 quantization of E and W gives max rel err ~4.5e-2 against
the 2e-2 gate (verified numerically), and correction passes erase the 2x
rate gain. PSUM accumulates in fp32; end-to-end rel err ~4e-4.
"""

import numpy as np

B, S, V, D = 8, 4096, 50257, 1024
CAT_DIMS = (1536, 1024, 512, 256)
NAMES = ("high", "mid", "low", "special")
N_CORES = 8
N_DCOL = D // 128                      # 8
ORDER = ("special", "low", "mid", "high")      # smallest tables first
MAX_MP = 1024                          # SBUF cap; excess falls back to host

_CACHE = {}
LAST_EXEC_NS = None
LAST_RESULTS = None


def _build_bass(mp):
    from contextlib import ExitStack
    import concourse.bacc as bacc
    import concourse.mybir as mybir
    import concourse.tile as tile

    nc = bacc.Bacc("TRN2", target_bir_lowering=False, debug=False,
                   num_devices=N_CORES)
    f16 = mybir.dt.float16
    f32 = mybir.dt.float32
    ident = mybir.ActivationFunctionType.Identity
    dims = dict(zip(NAMES, CAT_DIMS))
    c0n = min(512, mp)
    c1n = mp - c0n
    chunks = [(0, c0n)] + ([(c0n, c1n)] if c1n else [])

    xt_d, w_d, yt_d = {}, {}, {}
    for nm in NAMES:
        nk = dims[nm] // 128
        # inputs come pre-packed in SBUF partition layout
        xt_d[nm] = nc.dram_tensor(f"xt_{nm}", [128, nk * mp], f16,
                                  kind="ExternalInput")
        w_d[nm] = nc.dram_tensor(f"w_{nm}", [128, nk * D], f16,
                                 kind="ExternalInput")
        yt_d[nm] = nc.dram_tensor(f"yt_{nm}", [D, mp], f16,
                                  kind="ExternalOutput")
    # bias packed host-side as [128, 4*8]: column c*8+j holds b_c[j*128:(j+1)*128]
    bias_d = nc.dram_tensor("bias", [128, len(NAMES) * N_DCOL], f32,
                            kind="ExternalInput")

    with tile.TileContext(nc) as tc, ExitStack() as ctx:
        wpool = ctx.enter_context(tc.tile_pool(name="w", bufs=1))
        xpool = ctx.enter_context(tc.tile_pool(name="x", bufs=4))
        # one buffer per output j-block so deferred DMAs never recycle
        opool = ctx.enter_context(tc.tile_pool(name="o", bufs=32))
        bpool = ctx.enter_context(tc.tile_pool(name="b", bufs=1))
        ppool = ctx.enter_context(tc.tile_pool(name="p", bufs=3, space="PSUM"))

        bias_t = bpool.tile([128, len(NAMES) * N_DCOL], f32)

        # PE warm-up: dummy matmuls on a zeroed tile while the first real
        # inputs stream in, releasing the HAM clock-gate (2.4 GHz by ~3us of
        # PE activity). Short because two input queues land data by ~9.5us.
        warm = bpool.tile([128, 640], f16, name="warm")
        nc.vector.memset(warm[:], 0.0)
        wps = ppool.tile([128, 512], f32, tag="warmps", name="warmps", bufs=1)
        for r in range(6):
            nc.tensor.matmul(wps[:], warm[:, :128], warm[:, 128:640],
                             start=(r == 0), stop=(r == 5))

        w_t, x_t = {}, {}
        for nm in ORDER:
            nk = dims[nm] // 128
            w_t[nm] = wpool.tile([128, nk * D], f16, tag=f"w_{nm}",
                                 name=f"w_{nm}_sb")
            x_t[nm] = xpool.tile([128, 12 * mp], f16, tag="xslab",
                                 name=f"x_{nm}")

        # Input streams, strictly need-ordered and load-balanced across the
        # two HWDGE queues (~6.0MB sync / ~5.3MB scalar): early-needed bytes
        # lead on both queues so neither starves the PE. No input rides the
        # gpsimd queue (its outputs are deferred, below, to keep the early
        # window input-only).
        nc.sync.dma_start(w_t["special"][:, :D], w_d["special"].ap()[:, :D])
        nc.sync.dma_start(w_t["special"][:, D:2 * D],
                          w_d["special"].ap()[:, D:2 * D])
        nc.sync.dma_start(bias_t[:], bias_d.ap())
        nc.sync.dma_start(w_t["low"][:], w_d["low"].ap())
        nc.sync.dma_start(x_t["mid"][:, :8 * mp], xt_d["mid"].ap())
        nc.sync.dma_start(w_t["high"][:], w_d["high"].ap())

        nc.scalar.dma_start(x_t["special"][:, :mp],
                            xt_d["special"].ap()[:, :mp])
        nc.scalar.dma_start(x_t["special"][:, mp:2 * mp],
                            xt_d["special"].ap()[:, mp:2 * mp])
        nc.scalar.dma_start(x_t["low"][:, :4 * mp], xt_d["low"].ap())
        nc.scalar.dma_start(w_t["mid"][:], w_d["mid"].ap())
        nc.scalar.dma_start(x_t["high"][:, :12 * mp], xt_d["high"].ap())

        deferred = []   # (dram row AP, o_t) for special/low output blocks
        for nm in ORDER:
            ci = NAMES.index(nm)
            nk = dims[nm] // 128
            for j in range(N_DCOL):
                pss = [ppool.tile([128, 512], f32, tag=f"acc{q}", name=f"ps{q}")
                       for q in range(len(chunks))]
                for k in range(nk):
                    # one stationary load of W[k-block, j-block] serves both
                    # token chunks
                    for q, (c0, n) in enumerate(chunks):
                        nc.tensor.matmul(
                            pss[q][:, :n],
                            w_t[nm][:, k * D + j * 128: k * D + (j + 1) * 128],
                            x_t[nm][:, k * mp + c0: k * mp + c0 + n],
                            start=(k == 0),
                            stop=(k == nk - 1),
                        )
                o_t = opool.tile([128, mp], f16, tag="ostage")
                bias_ap = bias_t[:, ci * N_DCOL + j: ci * N_DCOL + j + 1]
                # split the PSUM drain across two engines so it never paces PE
                nc.vector.tensor_scalar_add(o_t[:, :c0n], pss[0][:, :c0n],
                                            bias_ap)
                if c1n:
                    nc.scalar.activation(o_t[:, c0n:mp], pss[1][:, :c1n],
                                         ident, bias=bias_ap)
                r0, r1 = j * 128, (j + 1) * 128
                if nm in ("special", "low"):
                    # defer: ship only after mid j0, so the input stream has
                    # the HBM/DMA budget to itself during the early window
                    deferred.append((yt_d[nm].ap()[r0:r1, :], o_t))
                elif nm == "mid":
                    nc.gpsimd.dma_start(yt_d[nm].ap()[r0:r1, :], o_t[:])
                    if j == 0:
                        for row, ot in deferred:
                            nc.gpsimd.dma_start(row, ot[:])
                        deferred = []
                elif j < N_DCOL - 1 or not c1n:
                    # last category exits via the by-then-idle sync HWDGE
                    nc.sync.dma_start(yt_d[nm].ap()[r0:r1, :], o_t[:])
                else:
                    # tail: ship each chunk as soon as its drain lands
                    nc.sync.dma_start(yt_d[nm].ap()[r0:r1, :c0n], o_t[:, :c0n])
                    nc.sync.dma_start(yt_d[nm].ap()[r0:r1, c0n:mp],
                                      o_t[:, c0n:mp])
    nc.compile()
    return nc


def _get_nc(mp):
    if mp not in _CACHE:
        _CACHE[mp] = _build_bass(mp)
    return _CACHE[mp]


def _pack_sbuf_layout(a2d):
    """[nk*128, F] -> [128, nk*F] (SBUF partition-major, contiguous)."""
    nk = a2d.shape[0] // 128
    f = a2d.shape[1]
    return np.ascontiguousarray(
        a2d.reshape(nk, 128, f).transpose(1, 0, 2).reshape(128, nk * f)
    )


def kernel(_profile=False, **inputs):
    global LAST_EXEC_NS, LAST_RESULTS
    from concourse.bass_utils import run_bass_kernel_spmd

    token_ids = np.asarray(inputs["token_ids"]).astype(np.int64)
    cat_table = np.asarray(inputs["cat_table"]).astype(np.int64)
    emb = {nm: np.asarray(inputs[f"emb_{nm}"], dtype=np.float32) for nm in NAMES}
    W = {nm: np.asarray(inputs[f"W_{nm}"], dtype=np.float32) for nm in NAMES}
    bvec = {nm: np.asarray(inputs[f"b_{nm}"], dtype=np.float32) for nm in NAMES}

    W16 = {nm: _pack_sbuf_layout(W[nm].astype(np.float16)) for nm in NAMES}
    bias_packed = np.ascontiguousarray(
        np.concatenate([bvec[nm].reshape(N_DCOL, 128).T for nm in NAMES], axis=1),
        dtype=np.float32)

    tok_flat = token_ids.reshape(-1)          # [32768]
    uniq, inv = np.unique(tok_flat, return_inverse=True)
    ucats = cat_table[uniq]                   # [n_uniq]

    # Unique-token routing: each category's unique-token list is split evenly
    # across the 8 cores (tables are replicated). M_PAD is sized from the
    # actual per-category counts so there is no overflow for this input; a
    # host fallback guards pathological distributions that exceed MAX_MP.
    counts = [(ucats == ci).sum() for ci in range(len(NAMES))]
    mp = int(max(512 + 2, -(-max(counts) // N_CORES)))
    mp += mp % 2
    mp = min(mp, MAX_MP)

    groups = {}     # (core, nm) -> unique-token indices (into uniq)
    overflow = []   # (nm, unique-token indices beyond total capacity)
    for ci, nm in enumerate(NAMES):
        pos = np.nonzero(ucats == ci)[0]
        if len(pos) > N_CORES * mp:
            overflow.append((nm, pos[N_CORES * mp:]))
            pos = pos[:N_CORES * mp]
        for core in range(N_CORES):
            groups[(core, nm)] = pos[core * mp:(core + 1) * mp]

    in_maps = []
    for core in range(N_CORES):
        im = {"bias": bias_packed}
        for ci, (nm, d) in enumerate(zip(NAMES, CAT_DIMS)):
            pos = groups[(core, nm)]
            n = len(pos)
            X = np.zeros((mp, d), np.float16)
            if n:
                X[:n] = emb[nm][uniq[pos]]
            # [mp, d] -> K-major [d, mp] -> SBUF layout [128, nk*mp]
            nk = d // 128
            im[f"xt_{nm}"] = np.ascontiguousarray(
                X.reshape(mp, nk, 128).transpose(2, 1, 0).reshape(128, nk * mp)
            )
            im[f"w_{nm}"] = W16[nm]
        in_maps.append(im)

    nc = _get_nc(mp)
    res = run_bass_kernel_spmd(nc, in_maps, list(range(N_CORES)),
                               trace=bool(_profile))
    LAST_EXEC_NS = res.exec_time_ns
    LAST_RESULTS = res

    out_u = np.empty((len(uniq), D), np.float32)
    for core in range(N_CORES):
        for nm in NAMES:
            pos = groups[(core, nm)]
            n = len(pos)
            if n:
                yt = res.results[core][f"yt_{nm}"]     # [D, mp] fp16
                out_u[pos] = yt[:, :n].T.astype(np.float32)
    # pathological excess beyond 8*mp unique tokens in one category: host
    for nm, pos in overflow:
        rows = emb[nm][uniq[pos]]
        out_u[pos] = rows @ W[nm] + bvec[nm]

    return out_u[inv].reshape(B, S, D)


# revision 10
# speedup vs baseline: 1.1781x; 1.0289x over previous
"""Bass/Trainium2 kernel for nn_BespokeEmbedding (moe_routing).

Strategy (unique-token data-parallel across 8 NeuronCores):
  - Host dedups the 32768 tokens to their ~24k unique ids (output rows are
    identical for repeated ids), routes the unique tokens into per-category
    groups split evenly across the cores, and gathers each group's embedding
    rows into a contraction-major fp16 activation block pre-packed into the
    SBUF partition layout. M_PAD (per-core per-category padded group size)
    is derived from the actual counts (~754 vs 1024 without dedup), so the
    matmul stream shrinks ~35%.
  - Each core runs one Bass/Tile kernel: for every category (smallest first,
    streamed just-in-time), a dense fp16 matmul Y_c^T = W_c^T @ X_c^T
    accumulated over 128-row K tiles in PSUM, one stationary-weight load
    serving both token chunks, bias-add fused into the PSUM drain (split
    across Vector and Scalar engines), result streamed back as fp16.
  - Inputs stream on two HWDGE queues in parallel (weights on sync, X on
    scalar) plus the tail half of W_high on the gpsimd queue, because the
    deduped compute (~65us) outruns a single ~290 GB/s input queue.
  - Host scatters unique rows back to all token positions (inverse of the
    dedup) and returns the full [8, 4096, 1024] float32 output.

fp16 runs the PE at 1 cycle/row; fp8 double-pumping was evaluated and
rejected: e4m3 quantization of E and W gives max rel err ~4.5e-2 against
the 2e-2 gate (verified numerically), and correction passes erase the 2x
rate gain. PSUM accumulates in fp32; end-to-end rel err ~4e-4.
"""

import numpy as np

B, S, V, D = 8, 4096, 50257, 1024
CAT_DIMS = (1536, 1024, 512, 256)
NAMES = ("high", "mid", "low", "special")
N_CORES = 8
N_DCOL = D // 128                      # 8
ORDER = ("special", "low", "mid", "high")      # smallest tables first
MAX_MP = 1024                          # SBUF cap; excess falls back to host

_CACHE = {}
LAST_EXEC_NS = None
LAST_RESULTS = None


def _build_bass(mp):
    from contextlib import ExitStack
    import concourse.bacc as bacc
    import concourse.mybir as mybir
    import concourse.tile as tile

    nc = bacc.Bacc("TRN2", target_bir_lowering=False, debug=False,
                   num_devices=N_CORES)
    f16 = mybir.dt.float16
    f32 = mybir.dt.float32
    ident = mybir.ActivationFunctionType.Identity
    dims = dict(zip(NAMES, CAT_DIMS))
    c0n = min(512, mp)
    c1n = mp - c0n
    chunks = [(0, c0n)] + ([(c0n, c1n)] if c1n else [])

    xt_d, w_d, yt_d = {}, {}, {}
    for nm in NAMES:
        nk = dims[nm] // 128
        # inputs come pre-packed in SBUF partition layout; W for the larger
        # categories is split into j-column halves (a: out-cols 0-511,
        # b: 512-1023 per k-block) so the b-half's DMA deadline slides to
        # midway through that category's j-loop
        xt_d[nm] = nc.dram_tensor(f"xt_{nm}", [128, nk * mp], f16,
                                  kind="ExternalInput")
        if nm == "special":
            w_d[nm] = nc.dram_tensor(f"w_{nm}", [128, nk * D], f16,
                                     kind="ExternalInput")
        else:
            w_d[nm] = (nc.dram_tensor(f"w_{nm}_a", [128, nk * (D // 2)], f16,
                                      kind="ExternalInput"),
                       nc.dram_tensor(f"w_{nm}_b", [128, nk * (D // 2)], f16,
                                      kind="ExternalInput"))
        yt_d[nm] = nc.dram_tensor(f"yt_{nm}", [D, mp], f16,
                                  kind="ExternalOutput")
    # bias packed host-side as [128, 4*8]: column c*8+j holds b_c[j*128:(j+1)*128]
    bias_d = nc.dram_tensor("bias", [128, len(NAMES) * N_DCOL], f32,
                            kind="ExternalInput")

    with tile.TileContext(nc) as tc, ExitStack() as ctx:
        wpool = ctx.enter_context(tc.tile_pool(name="w", bufs=1))
        xpool = ctx.enter_context(tc.tile_pool(name="x", bufs=4))
        # one buffer per output j-block so deferred DMAs never recycle
        opool = ctx.enter_context(tc.tile_pool(name="o", bufs=32))
        bpool = ctx.enter_context(tc.tile_pool(name="b", bufs=1))
        ppool = ctx.enter_context(tc.tile_pool(name="p", bufs=3, space="PSUM"))

        bias_t = bpool.tile([128, len(NAMES) * N_DCOL], f32)

        # PE warm-up: dummy matmuls on a zeroed tile while the first real
        # inputs stream in, releasing the HAM clock-gate (2.4 GHz by ~3us of
        # PE activity). Short because two input queues land data by ~9.5us.
        warm = bpool.tile([128, 640], f16, name="warm")
        nc.vector.memset(warm[:], 0.0)
        wps = ppool.tile([128, 512], f32, tag="warmps", name="warmps", bufs=1)
        for r in range(6):
            nc.tensor.matmul(wps[:], warm[:, :128], warm[:, 128:640],
                             start=(r == 0), stop=(r == 5))

        w_t, x_t = {}, {}
        for nm in ORDER:
            nk = dims[nm] // 128
            if nm == "special":
                w_t[nm] = wpool.tile([128, nk * D], f16, tag=f"w_{nm}",
                                     name=f"w_{nm}_sb")
            else:
                w_t[nm] = (wpool.tile([128, nk * (D // 2)], f16,
                                      tag=f"w_{nm}_a", name=f"w_{nm}_a_sb"),
                           wpool.tile([128, nk * (D // 2)], f16,
                                      tag=f"w_{nm}_b", name=f"w_{nm}_b_sb"))
            x_t[nm] = xpool.tile([128, 12 * mp], f16, tag="xslab",
                                 name=f"x_{nm}")

        # Input streams: W only on the sync HWDGE queue, X only on the
        # scalar HWDGE queue (the compile-time scheduler preserves
        # single-kind streams in emission order; mixing W into the X queue
        # got reordered). Each stream is strictly deadline-ordered; with the
        # j-half W split every deadline is met at ~180 B/ns per queue.
        nc.sync.dma_start(w_t["special"][:, :D], w_d["special"].ap()[:, :D])
        nc.sync.dma_start(w_t["special"][:, D:2 * D],
                          w_d["special"].ap()[:, D:2 * D])
        nc.sync.dma_start(bias_t[:], bias_d.ap())
        for nm in ("low", "mid", "high"):
            for h in (0, 1):
                nc.sync.dma_start(w_t[nm][h][:], w_d[nm][h].ap())

        nc.scalar.dma_start(x_t["special"][:, :mp],
                            xt_d["special"].ap()[:, :mp])
        nc.scalar.dma_start(x_t["special"][:, mp:2 * mp],
                            xt_d["special"].ap()[:, mp:2 * mp])
        nc.scalar.dma_start(x_t["low"][:, :4 * mp], xt_d["low"].ap())
        nc.scalar.dma_start(x_t["mid"][:, :8 * mp], xt_d["mid"].ap())
        nc.scalar.dma_start(x_t["high"][:, :12 * mp], xt_d["high"].ap())

        deferred = []   # (dram row AP, o_t) for special/low output blocks
        for nm in ORDER:
            ci = NAMES.index(nm)
            nk = dims[nm] // 128
            for j in range(N_DCOL):
                pss = [ppool.tile([128, 512], f32, tag=f"acc{q}", name=f"ps{q}")
                       for q in range(len(chunks))]
                hj, jj = divmod(j, 4)
                for k in range(nk):
                    if nm == "special":
                        wsrc = w_t[nm][:, k * D + j * 128:
                                       k * D + (j + 1) * 128]
                    else:
                        wsrc = w_t[nm][hj][:, k * (D // 2) + jj * 128:
                                           k * (D // 2) + (jj + 1) * 128]
                    # one stationary load of W[k-block, j-block] serves both
                    # token chunks
                    for q, (c0, n) in enumerate(chunks):
                        nc.tensor.matmul(
                            pss[q][:, :n],
                            wsrc,
                            x_t[nm][:, k * mp + c0: k * mp + c0 + n],
                            start=(k == 0),
                            stop=(k == nk - 1),
                        )
                o_t = opool.tile([128, mp], f16, tag="ostage")
                bias_ap = bias_t[:, ci * N_DCOL + j: ci * N_DCOL + j + 1]
                # split the PSUM drain across two engines so it never paces PE
                nc.vector.tensor_scalar_add(o_t[:, :c0n], pss[0][:, :c0n],
                                            bias_ap)
                if c1n:
                    nc.scalar.activation(o_t[:, c0n:mp], pss[1][:, :c1n],
                                         ident, bias=bias_ap)
                r0, r1 = j * 128, (j + 1) * 128
                if nm in ("special", "low"):
                    # defer: ship only after mid j0, so the input stream has
                    # the HBM/DMA budget to itself during the early window
                    deferred.append((yt_d[nm].ap()[r0:r1, :], o_t))
                elif nm == "mid":
                    nc.gpsimd.dma_start(yt_d[nm].ap()[r0:r1, :], o_t[:])
                    if j == 0:
                        for row, ot in deferred:
                            nc.gpsimd.dma_start(row, ot[:])
                        deferred = []
                elif j < N_DCOL - 1 or not c1n:
                    # last category exits via the by-then-idle sync HWDGE
                    nc.sync.dma_start(yt_d[nm].ap()[r0:r1, :], o_t[:])
                else:
                    # tail: ship each chunk as soon as its drain lands
                    nc.sync.dma_start(yt_d[nm].ap()[r0:r1, :c0n], o_t[:, :c0n])
                    nc.sync.dma_start(yt_d[nm].ap()[r0:r1, c0n:mp],
                                      o_t[:, c0n:mp])
    nc.compile()
    return nc


def _get_nc(mp):
    if mp not in _CACHE:
        _CACHE[mp] = _build_bass(mp)
    return _CACHE[mp]


def _pack_sbuf_layout(a2d):
    """[nk*128, F] -> [128, nk*F] (SBUF partition-major, contiguous)."""
    nk = a2d.shape[0] // 128
    f = a2d.shape[1]
    return np.ascontiguousarray(
        a2d.reshape(nk, 128, f).transpose(1, 0, 2).reshape(128, nk * f)
    )


def kernel(_profile=False, **inputs):
    global LAST_EXEC_NS, LAST_RESULTS
    from concourse.bass_utils import run_bass_kernel_spmd

    token_ids = np.asarray(inputs["token_ids"]).astype(np.int64)
    cat_table = np.asarray(inputs["cat_table"]).astype(np.int64)
    emb = {nm: np.asarray(inputs[f"emb_{nm}"], dtype=np.float32) for nm in NAMES}
    W = {nm: np.asarray(inputs[f"W_{nm}"], dtype=np.float32) for nm in NAMES}
    bvec = {nm: np.asarray(inputs[f"b_{nm}"], dtype=np.float32) for nm in NAMES}

    W16 = {}
    for nm in NAMES:
        w16 = W[nm].astype(np.float16)
        if nm == "special":
            W16["w_special"] = _pack_sbuf_layout(w16)
        else:
            W16[f"w_{nm}_a"] = _pack_sbuf_layout(w16[:, :D // 2])
            W16[f"w_{nm}_b"] = _pack_sbuf_layout(w16[:, D // 2:])
    bias_packed = np.ascontiguousarray(
        np.concatenate([bvec[nm].reshape(N_DCOL, 128).T for nm in NAMES], axis=1),
        dtype=np.float32)

    tok_flat = token_ids.reshape(-1)          # [32768]
    uniq, inv = np.unique(tok_flat, return_inverse=True)
    ucats = cat_table[uniq]                   # [n_uniq]

    # Unique-token routing: each category's unique-token list is split evenly
    # across the 8 cores (tables are replicated). M_PAD is sized from the
    # actual per-category counts so there is no overflow for this input; a
    # host fallback guards pathological distributions that exceed MAX_MP.
    counts = [(ucats == ci).sum() for ci in range(len(NAMES))]
    mp = int(max(512 + 2, -(-max(counts) // N_CORES)))
    mp += mp % 2
    mp = min(mp, MAX_MP)

    groups = {}     # (core, nm) -> unique-token indices (into uniq)
    overflow = []   # (nm, unique-token indices beyond total capacity)
    for ci, nm in enumerate(NAMES):
        pos = np.nonzero(ucats == ci)[0]
        if len(pos) > N_CORES * mp:
            overflow.append((nm, pos[N_CORES * mp:]))
            pos = pos[:N_CORES * mp]
        for core in range(N_CORES):
            groups[(core, nm)] = pos[core * mp:(core + 1) * mp]

    in_maps = []
    for core in range(N_CORES):
        im = {"bias": bias_packed}
        for ci, (nm, d) in enumerate(zip(NAMES, CAT_DIMS)):
            pos = groups[(core, nm)]
            n = len(pos)
            X = np.zeros((mp, d), np.float16)
            if n:
                X[:n] = emb[nm][uniq[pos]]
            # [mp, d] -> K-major [d, mp] -> SBUF layout [128, nk*mp]
            nk = d // 128
            im[f"xt_{nm}"] = np.ascontiguousarray(
                X.reshape(mp, nk, 128).transpose(2, 1, 0).reshape(128, nk * mp)
            )
        im.update(W16)
        in_maps.append(im)

    nc = _get_nc(mp)
    res = run_bass_kernel_spmd(nc, in_maps, list(range(N_CORES)),
                               trace=bool(_profile))
    LAST_EXEC_NS = res.exec_time_ns
    LAST_RESULTS = res

    out_u = np.empty((len(uniq), D), np.float32)
    for core in range(N_CORES):
        for nm in NAMES:
            pos = groups[(core, nm)]
            n = len(pos)
            if n:
                yt = res.results[core][f"yt_{nm}"]     # [D, mp] fp16
                out_u[pos] = yt[:, :n].T.astype(np.float32)
    # pathological excess beyond 8*mp unique tokens in one category: host
    for nm, pos in overflow:
        rows = emb[nm][uniq[pos]]
        out_u[pos] = rows @ W[nm] + bvec[nm]

    return out_u[inv].reshape(B, S, D)
